# revision 2
# baseline (speedup 1.0000x reference)
"""DeeperGCN layer v4: per-dst-node layout + compact per-half gather tables.

Two-stage gather: (1) build compact source tables tabA/tabB (the unique hn
rows each half of the tile range needs, <=32768 rows so int16 indexes reach
everything) with sequential-ish window gathers from the AllGather'd hnf;
(2) per-edge gathers read the compact table with a SINGLE all-valid call per
round group - no dual windows, no dummy-row traffic (~40% fewer descriptors).

Versus v1 (indicator-matmul): edges land at (partition = dst slot, free round),
so the per-node softmax sums are plain DVE tensor_reduce over the round axis —
no indicator build (~850 DVE insts/core) and no accumulation matmuls
(~1700 PE insts/core). Padding handled arithmetically: pad slots get
eattr = -1e30 (=> ex = exp(t*eps) exactly, mex = 0), and the host sends
per-slot pad counts so den is corrected by padc*exp(t*eps) in one op.

The int16 gather-index limit (32767 < NP=50176) is handled with two gather
calls per round group writing disjoint positions: call L gathers src<SPLIT
rows from hnf[0:SPLIT] (other positions = -1, skipped by HW, probe-verified
position-preserving), call H gathers the rest from hnf[SPLIT:].

LayerNorm rstd avoids ACT-table thrash (Ln set vs Exp set): ln(v) comes from
the float-bits approximation + one exp-based Newton step, so every ACT op in
the loop lives in the 'exp_and_others' table set -> no LoadActFuncSet per tile.
"""

import dataclasses
import numpy as np

import concourse.bass as bass
import concourse.bacc as bacc
import concourse.tile as tile
import concourse.mybir as mybir
from concourse.masks import make_identity

F32 = mybir.dt.float32
F32R = mybir.dt.float32r
BF16 = mybir.dt.bfloat16
I32 = mybir.dt.int32
I16 = mybir.dt.int16
AF = mybir.ActivationFunctionType
OP = mybir.AluOpType

EPS = 1e-7
BN_EPS = 1e-5
LN_EPS = 1e-5
DEN_EPS = 1e-16
NEG_BIG = -1e30

# ln(v) ~= C1*float(bits(v)) - C3  (|err| <= 0.0861*ln2), then 1 Newton step
LN_C1 = float(np.log(2.0) / (1 << 23))
LN_C3 = float((127.0 - 0.0430357) * np.log(2.0))


@dataclasses.dataclass
class Cfg:
    n_cores: int = 8
    H: int = 128
    NT: int = 49
    R: tuple = ()            # rounds per local tile (len NT, same on all cores)
    SPLIT: int = 32768
    GMAX: int = 8            # rounds per dma_gather call (1024 idx)
    scratch: int = 16384     # SWDGE descriptor carveout bytes/partition
    n_queues: int = 4
    stream_bufs: int = 3
    small_bufs: int = 3
    skip_collective: bool = False
    hw_repeat: int = 1
    apply_b1: bool = False
    apply_b2: bool = False
    apply_ln_affine: bool = False
    ablate: str = ""
    gp_add: bool = False     # s = hs+ea on gpsimd (frees DVE)
    gp_clamp: bool = False   # (unused)
    fold_reduce: bool = False  # bf16 pair-fold rounds before strided reduce
    single_packet: bool = True
    halves: tuple = ((0, 25), (25, 49))  # overwritten by host_prep
    U: tuple = ()            # padded unique-source count per half (<=32768)
    BASES: tuple = ()        # per-build-call hnf window base (len sum(U)//1024)

    @property
    def NPC(self):
        return self.NT * 128

    @property
    def NP(self):
        return self.NPC * self.n_cores

    @property
    def H2(self):
        return 2 * self.H

    @property
    def W(self):
        return int(sum(self.R))

    @property
    def Rmax(self):
        return int(max(self.R))


def build_gcn(cfg: Cfg):
    H, H2, NT = cfg.H, cfg.H2, cfg.NT
    NPC, NP = cfg.NPC, cfg.NP
    R_list = list(cfg.R)
    offs = np.concatenate([[0], np.cumsum(R_list)]).astype(int)
    W = cfg.W
    Rmax = cfg.Rmax
    sdt = BF16

    nc = bacc.Bacc("TRN2", target_bir_lowering=False, debug=False,
                   num_devices=cfg.n_cores, num_swdge_queues=cfg.n_queues,
                   dynamic_dma_scratch_size=cfg.scratch)

    # ---- I/O ----
    h_rows = nc.dram_tensor("h_rows", [NPC, H], F32, kind="ExternalInput").ap()
    bnw = nc.dram_tensor("bnw", [1, H], F32, kind="ExternalInput").ap()
    bnb = nc.dram_tensor("bnb", [1, H], F32, kind="ExternalInput").ap()
    bnm = nc.dram_tensor("bnm", [1, H], F32, kind="ExternalInput").ap()
    bnv = nc.dram_tensor("bnv", [1, H], F32, kind="ExternalInput").ap()
    t_sc = nc.dram_tensor("t_sc", [1, 1], F32, kind="ExternalInput").ap()
    W1 = nc.dram_tensor("W1", [H, H2], F32, kind="ExternalInput").ap()
    W2 = nc.dram_tensor("W2", [H2, H], F32, kind="ExternalInput").ap()
    if cfg.apply_b1:
        b1 = nc.dram_tensor("b1", [1, H2], F32, kind="ExternalInput").ap()
    if cfg.apply_b2:
        b2 = nc.dram_tensor("b2", [1, H], F32, kind="ExternalInput").ap()
    if cfg.apply_ln_affine:
        lnw = nc.dram_tensor("lnw", [1, H2], F32, kind="ExternalInput").ap()
        lnb = nc.dram_tensor("lnb", [1, H2], F32, kind="ExternalInput").ap()
    eattr = nc.dram_tensor("eattr", [128, W * 128], sdt,
                           kind="ExternalInput").ap()
    gidx = nc.dram_tensor("gidx", [128, W * 8], I16,
                          kind="ExternalInput").ap()
    UW = int(sum(cfg.U))
    bidx = nc.dram_tensor("bidx", [128, UW // 16], I16,
                          kind="ExternalInput").ap()
    degc = nc.dram_tensor("degc", [128, NT], F32, kind="ExternalInput").ap()
    out = nc.dram_tensor("out", [NPC, H], F32, kind="ExternalOutput").ap()

    hnbg = nc.dram_tensor("hnbg", [NPC, H], sdt).ap()
    hnf = nc.dram_tensor("hnf", [NP, H], sdt, addr_space="Shared").ap()
    tabs = [nc.dram_tensor(f"tab{h}", [int(cfg.U[h]), H], sdt).ap()
            for h in range(len(cfg.halves))]

    with tile.TileContext(nc) as tc:
        with tc.tile_pool(name="const", bufs=1) as cpool, \
             tc.tile_pool(name="colv", bufs=1) as colp:
            ident = cpool.tile([128, 128], F32)
            make_identity(nc, ident[:])
            # W1 with appended row-sums column (y1[:,H2] = row sums for LN mu)
            w1_st = cpool.tile([H, H2 + 2], F32, tag="w1_st")
            nc.gpsimd.memset(w1_st[:, H2:H2 + 2], 0.0)
            nc.sync.dma_start(w1_st[:, 0:H2], W1[:])
            nc.vector.tensor_reduce(w1_st[:, H2:H2 + 1], w1_st[:, 0:H2],
                                    mybir.AxisListType.X, OP.add)
            w1_sb = cpool.tile([H, H2 + 2], F32R, tag="w1_sb")
            nc.scalar.copy(w1_sb[:], w1_st[:])
            w2_st = cpool.tile([H2 // 2, 2 * H], F32, tag="w2_st")
            nc.sync.dma_start(w2_st[:, 0:H], W2[0:H, :])
            nc.sync.dma_start(w2_st[:, H:2 * H], W2[H:H2, :])
            w2_sb = cpool.tile([H2 // 2, 2 * H], F32R, tag="w2_sb")
            nc.scalar.copy(w2_sb[:], w2_st[:])
            w2a_sb = w2_sb[:, 0:H]
            w2b_sb = w2_sb[:, H:2 * H]

            # temperature columns
            t_c1 = colp.tile([1, 1], F32)
            nc.sync.dma_start(t_c1[:], t_sc[:])
            t_c = colp.tile([128, 1], F32)
            nc.gpsimd.partition_broadcast(t_c[:], t_c1[:])
            teps_c = colp.tile([128, 1], F32)
            nc.vector.tensor_scalar_mul(teps_c[:], t_c[:], float(EPS))
            etc_c = colp.tile([128, 1], F32)   # exp(t*eps)
            nc.scalar.activation(etc_c[:], teps_c[:], AF.Exp)
            bneps_r = colp.tile([1, 1], F32)
            nc.gpsimd.memset(bneps_r[:], float(BN_EPS))
            ones_c = colp.tile([1, 128], F32)
            nc.gpsimd.memset(ones_c[:], 1.0)
            netc_c = colp.tile([128, 1], F32)  # -exp(t*eps), ACT relu bias
            nc.vector.tensor_scalar_mul(netc_c[:], etc_c[:], -1.0)

            # den base per tile: pe2 = deg*etc + DEN_EPS
            # (den = sum relu(ex-etc) + deg*etc; pads contribute 0)
            degc_sb = cpool.tile([128, NT], F32, tag="degc_sb")
            nc.sync.dma_start(degc_sb[:], degc[:])
            pe2 = cpool.tile([128, NT], F32, tag="pe2")
            nc.vector.tensor_scalar(pe2[:], degc_sb[:], etc_c[:],
                                    float(DEN_EPS), OP.mult, OP.add)

            # bn affine: a = bnw*rsqrt(bnv+eps); c = bnb - bnm*a  (one-time,
            # Ln/Exp here is fine -- happens before the steady-state loop)
            bnw_r = colp.tile([1, H], F32, tag="bnw_r")
            nc.sync.dma_start(bnw_r[:], bnw[:])
            bnb_r = colp.tile([1, H], F32, tag="bnb_r")
            nc.sync.dma_start(bnb_r[:], bnb[:])
            bnm_r = colp.tile([1, H], F32, tag="bnm_r")
            nc.sync.dma_start(bnm_r[:], bnm[:])
            bnv_r = colp.tile([1, H], F32, tag="bnv_r")
            nc.sync.dma_start(bnv_r[:], bnv[:])
            lv_r = colp.tile([1, H], F32, tag="lv_r")
            nc.scalar.activation(lv_r[:], bnv_r[:], AF.Ln, bias=bneps_r[:])
            rs_r = colp.tile([1, H], F32, tag="rs_r")
            nc.scalar.activation(rs_r[:], lv_r[:], AF.Exp, scale=-0.5)
            a_r = colp.tile([1, H], F32, tag="a_r")
            nc.vector.tensor_mul(a_r[:], bnw_r[:], rs_r[:])
            ma_r = colp.tile([1, H], F32, tag="ma_r")
            nc.vector.tensor_mul(ma_r[:], bnm_r[:], a_r[:])
            c_r = colp.tile([1, H], F32, tag="c_r")
            nc.vector.tensor_sub(c_r[:], bnb_r[:], ma_r[:])

            with tc.tile_pool(name="bc_ps", bufs=2, space="PSUM") as bcps:
                def bcast(row_ap, width, nm):
                    ps = bcps.tile([128, width], F32, tag=f"bc_{nm}")
                    sb = cpool.tile([128, width], F32, tag=f"bcsb_{nm}")
                    nc.tensor.matmul(ps[:], lhsT=ones_c[:], rhs=row_ap,
                                     start=True, stop=True)
                    nc.scalar.copy(sb[:], ps[:])
                    return sb

                a_b = bcast(a_r[:], H, "a")
                c_b = bcast(c_r[:], H, "c")

                def bcast_dram(dram_row, width, nm):
                    row = colp.tile([1, width], F32, tag=f"bcrow_{nm}")
                    nc.sync.dma_start(row[:], dram_row)
                    return bcast(row[:], width, nm)

                b1_b = bcast_dram(b1[:], H2, "b1") if cfg.apply_b1 else None
                b2_b = bcast_dram(b2[:], H, "b2") if cfg.apply_b2 else None
                lnw_b = (bcast_dram(lnw[:], H2, "lnw")
                         if cfg.apply_ln_affine else None)
                lnb_b = (bcast_dram(lnb[:], H2, "lnb")
                         if cfg.apply_ln_affine else None)

            # persistent node data
            xall = cpool.tile([128, NPC], F32, tag="xall")
            hnall = cpool.tile([128, NPC], F32, tag="hnall")

          # whole-kernel repeat (timing amplification: dispatch floor cancels
          # in (t_K - t_1)/(K-1) marginal measurements)
          for rep in range(cfg.hw_repeat):
            # ---- phase A: hn = relu(a*h + c) ----
            with tc.tile_pool(name="pa", bufs=4) as pa:
                for t in range(NT):
                    sl = slice(t * 128, (t + 1) * 128)
                    nc.sync.dma_start(xall[:, sl], h_rows[sl, :])
                    u = pa.tile([128, 128], F32, tag="u")
                    nc.vector.tensor_mul(u[:], xall[:, sl], a_b[:])
                    v = pa.tile([128, 128], F32, tag="v")
                    nc.vector.tensor_add(v[:], u[:], c_b[:])
                    nc.scalar.activation(hnall[:, sl], v[:], AF.Relu)
                    hng = pa.tile([128, 128], sdt, tag="hng")
                    nc.vector.tensor_copy(hng[:], hnall[:, sl])
                    nc.scalar.dma_start(hnbg[sl, :], hng[:])

            # zero dummy rows at both ends of the gather table
            zrow = colp.tile([1, H], sdt, tag="zrow")
            nc.gpsimd.memset(zrow[:], 0.0)
            nc.scalar.dma_start(hnf[0:1, :], zrow[:])
            nc.scalar.dma_start(hnf[NP + 1:NP + 2, :], zrow[:])
            # ---- AllGather hn (into rows [1, NP+1)) ----
            if cfg.skip_collective:
                nc.sync.dma_start(hnf[1:NPC + 1, :], hnbg[:])
            else:
                nc.gpsimd.collective_compute(
                    "AllGather",
                    OP.bypass,
                    ins=[hnbg[:]],
                    outs=[hnf[1:NP + 1, :]],
                    replica_groups=[list(range(cfg.n_cores))],
                )

            # ---- phase B ----
            with tc.tile_pool(name="stream", bufs=cfg.stream_bufs) as sp, \
                 tc.tile_pool(name="expool", bufs=2) as xp, \
                 tc.tile_pool(name="small", bufs=cfg.small_bufs) as smp, \
                 tc.tile_pool(name="ps_tr", bufs=2, space="PSUM") as ps_tr, \
                 tc.tile_pool(name="ps_y", bufs=2, space="PSUM") as ps_y:
              qn = [0]
              if True:
                for t in range(NT):
                    R = R_list[t]
                    off = int(offs[t])
                    sl = slice(t * 128, (t + 1) * 128)
                    CE = R * 128
                    ea = sp.tile([128, Rmax * 128], sdt, tag="ea")
                    nc.sync.dma_start(ea[:, 0:CE],
                                      eattr[:, off * 128:(off + R) * 128])
                    ixt = smp.tile([128, Rmax * 16], I16, tag="ixt")
                    nc.scalar.dma_start(ixt[:, 0:R * 16],
                                        gidx[:, off * 16:(off + R) * 16])
                    hs = sp.tile([128, Rmax * 128], sdt, tag="hs")
                    hs3 = hs[:, 0:CE].rearrange("p (j c) -> p j c", c=128)
                    hsh = sp.tile([128, Rmax * 128], sdt, tag="hsh")
                    hsh3 = hsh[:, 0:CE].rearrange("p (j c) -> p j c", c=128)
                    HBASE = NP + 2 - 32768
                    if "gather" not in cfg.ablate:
                        for g0 in range(0, R, cfg.GMAX):
                            gr = min(cfg.GMAX, R - g0)
                            nc.gpsimd.dma_gather(
                                out_ap=hs3[:, g0:g0 + gr, :],
                                in_ap=hnf[0:32768, :],
                                idxs_ap=ixt[:, g0 * 8:(g0 + gr) * 8],
                                num_idxs=gr * 128,
                                num_idxs_reg=gr * 128,
                                elem_size=H,
                                queue_num=qn[0] % cfg.n_queues,
                            )
                            qn[0] += 1
                            nc.gpsimd.dma_gather(
                                out_ap=hsh3[:, g0:g0 + gr, :],
                                in_ap=hnf[HBASE:NP + 2, :],
                                idxs_ap=ixt[:, (R + g0) * 8:(R + g0 + gr) * 8],
                                num_idxs=gr * 128,
                                num_idxs_reg=gr * 128,
                                elem_size=H,
                                queue_num=qn[0] % cfg.n_queues,
                            )
                            qn[0] += 1
                    else:
                        nc.sync.dma_start(hs[:, 0:CE],
                                          eattr[:, off * 128:(off + R) * 128])
                        nc.gpsimd.memset(hsh[:, 0:CE], 0.0)
                    # s = (hsL + hsH) + ea; exactly one of hsL/hsH holds the
                    # real row, the other a zero dummy row
                    nc.vector.tensor_add(hs[:, 0:CE], hs[:, 0:CE],
                                         hsh[:, 0:CE])
                    if cfg.gp_add:
                        nc.gpsimd.tensor_add(hs[:, 0:CE], hs[:, 0:CE],
                                             ea[:, 0:CE])
                    else:
                        nc.vector.tensor_add(hs[:, 0:CE], hs[:, 0:CE],
                                             ea[:, 0:CE])
                    # ex = max(exp(t*s + t*eps), exp(t*eps))
                    ex = xp.tile([128, Rmax * 128], sdt, tag="ex")
                    exv = ex[:, 0:CE]
                    nc.scalar.activation(
                        exv.rearrange("p (j c) -> p j c", c=128), hs3,
                        AF.Exp, scale=t_c[:], bias=teps_c[:])
                    # mex = relu(s)*ex_raw (in place into hs; the etc clamp
                    # only binds where relu(s)=0, so raw ex is exact here)
                    nc.vector.scalar_tensor_tensor(
                        hs[:, 0:CE], hs[:, 0:CE], 0.0, exv, OP.max, OP.mult)
                    # exm = relu(ex - etc) on ACT (in place); den recovers as
                    # sum(exm) + deg*etc via pe2
                    nc.scalar.activation(exv, exv, AF.Relu, bias=netc_c[:])
                    # den/num via strided reduce over rounds
                    nd = smp.tile([128, 256], F32, tag="nd")
                    ex_r = ex[:, 0:CE].rearrange("p (j c) -> p c j", c=128)
                    mex_r = hs[:, 0:CE].rearrange("p (j c) -> p c j", c=128)
                    nc.vector.tensor_reduce(nd[:, 0:128], ex_r,
                                            mybir.AxisListType.X, OP.add)
                    nc.vector.tensor_reduce(nd[:, 128:256], mex_r,
                                            mybir.AxisListType.X, OP.add)
                    # d1 = sum(exm) + deg*etc + 1e-16; num = eps*d1 + mex_sum
                    d1 = smp.tile([128, 128], F32, tag="d1")
                    nc.vector.tensor_scalar(d1[:], nd[:, 0:128],
                                            pe2[:, t:t + 1], None, OP.add)
                    num = smp.tile([128, 128], F32, tag="num")
                    nc.vector.scalar_tensor_tensor(num[:], d1[:], float(EPS),
                                                   nd[:, 128:256],
                                                   OP.mult, OP.add)
                    rden = smp.tile([128, 128], F32, tag="rden")
                    nc.vector.reciprocal_approx_fast(rden[:], d1[:])
                    agg = smp.tile([128, 128], F32, tag="agg")
                    nc.vector.tensor_mul(agg[:], num[:], rden[:])
                    aggx = smp.tile([128, 128], F32, tag="aggx")
                    nc.vector.tensor_add(aggx[:], agg[:], hnall[:, sl])
                    # ---- MLP ----
                    tps = ps_tr.tile([128, 128], F32, tag="tps")
                    nc.tensor.transpose(tps[:], aggx[:], ident[:])
                    aggxT = smp.tile([128, 128], F32R, tag="aggxT")
                    nc.scalar.copy(aggxT[:], tps[:])
                    y1 = ps_y.tile([128, H2 + 2], F32, tag="y1")
                    nc.tensor.matmul(y1[:], lhsT=aggxT[:], rhs=w1_sb[:],
                                     start=True, stop=True)
                    if cfg.apply_b1:
                        y1s = smp.tile([128, H2], F32, tag="y1s")
                        nc.vector.tensor_add(y1s[:], y1[:, 0:H2], b1_b[:])
                        sums = smp.tile([128, 1], F32, tag="sums")
                        nc.vector.tensor_reduce(sums[:], y1s[:],
                                                mybir.AxisListType.X, OP.add)
                        y1v = y1s[:]
                        sums_v = sums[:]
                    else:
                        sums_v = y1[:, H2:H2 + 1]
                        y1v = y1[:, 0:H2]
                    sq = smp.tile([128, H2], F32, tag="sq")
                    sumsq = smp.tile([128, 1], F32, tag="sumsq")
                    nc.scalar.activation(sq[:], y1v, AF.Square,
                                         accum_out=sumsq[:])
                    mu = smp.tile([128, 1], F32, tag="mu")
                    nc.vector.tensor_scalar_mul(mu[:], sums_v, 1.0 / H2)
                    msq = smp.tile([128, 1], F32, tag="msq")
                    nc.vector.tensor_mul(msq[:], mu[:], mu[:])
                    var = smp.tile([128, 1], F32, tag="var")
                    nc.vector.scalar_tensor_tensor(var[:], sumsq[:], 1.0 / H2,
                                                   msq[:], OP.mult, OP.subtract)
                    # v = var + eps; rstd = exp(-0.5*ln(v)) with bit-trick ln
                    # + one exp Newton step (keeps ACT in the Exp table set)
                    v_t = smp.tile([128, 1], F32, tag="v_t")
                    nc.vector.tensor_scalar_add(v_t[:], var[:], float(LN_EPS))
                    vf = smp.tile([128, 1], F32, tag="vf")
                    nc.vector.tensor_copy(vf[:], v_t[:].bitcast(I32))
                    z0 = smp.tile([128, 1], F32, tag="z0")
                    nc.vector.tensor_scalar(z0[:], vf[:], LN_C1, LN_C3,
                                            OP.mult, OP.subtract)
                    e_t = smp.tile([128, 1], F32, tag="e_t")
                    nc.scalar.activation(e_t[:], z0[:], AF.Exp, scale=-1.0)
                    w_t = smp.tile([128, 1], F32, tag="w_t")
                    nc.vector.tensor_mul(w_t[:], v_t[:], e_t[:])
                    z1 = smp.tile([128, 1], F32, tag="z1")
                    nc.vector.scalar_tensor_tensor(z1[:], w_t[:], -1.0,
                                                   z0[:], OP.add, OP.add)
                    rstd = smp.tile([128, 1], F32, tag="rstd")
                    nc.scalar.activation(rstd[:], z1[:], AF.Exp, scale=-0.5)
                    z = smp.tile([128, H2], F32, tag="z")
                    nc.vector.tensor_scalar(z[:], y1v, mu[:], rstd[:],
                                            OP.subtract, OP.mult)
                    if cfg.apply_ln_affine:
                        nc.vector.tensor_mul(z[:], z[:], lnw_b[:])
                        nc.vector.tensor_add(z[:], z[:], lnb_b[:])
                    yr = smp.tile([128, H2], F32, tag="yr")
                    nc.scalar.activation(yr[:], z[:], AF.Relu)
                    o_ps = ps_tr.tile([128, H], F32, tag="o_ps")
                    for half in range(2):
                        tph = ps_tr.tile([128, 128], F32, tag="tps")
                        nc.tensor.transpose(
                            tph[:], yr[:, half * 128:(half + 1) * 128],
                            ident[:])
                        yT = smp.tile([128, 128], F32R, tag="yT")
                        nc.scalar.copy(yT[:], tph[:])
                        nc.tensor.matmul(
                            o_ps[:], lhsT=yT[:],
                            rhs=(w2a_sb if half == 0 else w2b_sb),
                            start=(half == 0), stop=(half == 1))
                    osb = smp.tile([128, 128], F32, tag="osb")
                    nc.vector.tensor_add(osb[:], o_ps[:], xall[:, sl])
                    if cfg.apply_b2:
                        nc.vector.tensor_add(osb[:], osb[:], b2_b[:])
                    nc.scalar.dma_start(out[sl, :], osb[:])

    nc.compile()
    return nc


# ---------------- host-side prep ----------------

try:
    import ml_dtypes
    ml_bf16 = ml_dtypes.bfloat16
except ImportError:
    ml_bf16 = np.float32


def host_prep(h, edge_index, edge_attr, bn_weight, bn_bias, bn_mean, bn_var,
              t, W1, b1, ln_weight, ln_bias, W2, b2, n_cores=8,
              sort_rounds=True):
    h = np.asarray(h, np.float32)
    edge_index = np.asarray(edge_index).astype(np.int64)
    edge_attr = np.asarray(edge_attr, np.float32)
    N, H = h.shape
    E = edge_index.shape[1]
    SPLIT = 32512

    n_glob = int(np.ceil(N / 128))
    n_glob = int(np.ceil(n_glob / n_cores)) * n_cores
    NT = n_glob // n_cores
    NPC = NT * 128
    NP = NPC * n_cores

    src = edge_index[0]
    dst = edge_index[1]
    deg = np.bincount(dst, minlength=N).astype(np.int64)

    # nodes sorted by degree desc; rank blocks of 128 = global tiles;
    # global tile gt -> core gt%8, local tile gt//8 (consecutive blocks of 8
    # tiles share a local index => near-equal R across cores)
    order = np.argsort(-deg, kind="stable")
    rank = np.empty(N, np.int64)
    rank[order] = np.arange(N, dtype=np.int64)
    gt = rank // 128
    slot = rank % 128
    core = gt % n_cores
    ltile = gt // n_cores
    p_glob = core * NPC + ltile * 128 + slot        # row in hnf

    # R per local tile = max degree in its 8-tile group (>=1)
    sdeg = deg[order]
    R = np.empty(NT, np.int64)
    for i in range(NT):
        lo = i * 128 * n_cores
        hi = min(lo + 128 * n_cores, N)
        R[i] = max(1, int(sdeg[lo:hi].max()) if hi > lo else 1)
    offs = np.concatenate([[0], np.cumsum(R)]).astype(np.int64)
    W = int(offs[-1])

    # per-edge placement
    e_rank = rank[dst]
    e_core = core[dst]
    e_tile = ltile[dst]
    e_slot = slot[dst]
    # sort_rounds: within each node, order edges by source row so each
    # gather call's descriptors form semi-ascending address streams
    if sort_rounds:
        order_e = np.lexsort((p_glob[src], e_rank))
    else:
        order_e = np.argsort(e_rank, kind="stable")
    er_s = e_rank[order_e]
    starts = np.zeros(N, np.int64)
    cnt = np.bincount(e_rank, minlength=N)
    # starts per rank-group in the sorted order
    rank_cnt = np.bincount(er_s, minlength=N)
    rank_starts = np.zeros(N, np.int64)
    np.cumsum(rank_cnt[:-1], out=rank_starts[1:])
    r_e_s = np.arange(E, dtype=np.int64) - rank_starts[er_s]
    r_e = np.empty(E, np.int64)
    r_e[order_e] = r_e_s

    ec = e_core
    et = e_tile
    es = e_slot
    psrc = p_glob[src]

    # eattr padded layout [cores][128 slots, W*128], pads = NEG_BIG
    EA = np.full((n_cores, 128, W, H), np.float32(NEG_BIG), ml_bf16)
    EA[ec, es, offs[et] + r_e, :] = edge_attr.astype(ml_bf16)
    EA = EA.reshape(n_cores, 128, W * H)

    # gather idx [cores][16 wrap, W*16]: per tile, L block then H block
    # compact tables: tile ranges balanced by edge count so each part's
    # unique-source count stays under the int16 reach
    NPARTS = 3
    cumR = np.cumsum(R)
    bounds = [0]
    for pben in range(1, NPARTS):
        tgt = cumR[-1] * pben / NPARTS
        bounds.append(int(np.searchsorted(cumR, tgt) + 1))
    bounds.append(NT)
    halves = tuple((bounds[j], bounds[j + 1]) for j in range(NPARTS))
    q = r_e * 128 + es
    qrow = q % 16
    qcol = q // 16
    NPv = NPC * n_cores
    half_of_tile = np.zeros(NT, np.int64)
    for hidx, (a, b) in enumerate(halves):
        half_of_tile[a:b] = hidx
    e_half = half_of_tile[et]
    uniq = [[None] * len(halves) for _ in range(n_cores)]
    for c in range(n_cores):
        for hidx in range(len(halves)):
            m = (ec == c) & (e_half == hidx)
            u = np.unique(psrc[m])
            uniq[c][hidx] = u
    U = []
    for hidx in range(len(halves)):
        umax = max(len(uniq[c][hidx]) for c in range(n_cores))
        Uh = int(np.ceil(umax / 1024)) * 1024
        assert Uh <= 32768, f"half {hidx} unique {umax} exceeds int16 reach"
        U.append(Uh)
    # padded unique lists + per-call window bases (uniform across cores)
    upad = np.zeros((n_cores, sum(U)), np.int64)
    for c in range(n_cores):
        o = 0
        for hidx in range(len(halves)):
            u = uniq[c][hidx]
            upad[c, o:o + len(u)] = u
            upad[c, o + len(u):o + U[hidx]] = u[-1]
            o += U[hidx]
    BASES = []
    o = 0
    for hidx in range(len(halves)):
        for k in range(U[hidx] // 1024):
            lo_b = min(int(upad[c, o + k * 1024]) for c in range(n_cores))
            hi_b = max(int(upad[c, o + (k + 1) * 1024 - 1])
                       for c in range(n_cores))
            B = min(lo_b, NPv - 32768)
            assert hi_b - B <= 32767, (hidx, k, hi_b - B)
            BASES.append(B)
        o += U[hidx]
    # build-gather idx input (16-wrap, replicated to 128)
    UW = sum(U)
    b16 = np.zeros((n_cores, 16, UW // 16), np.int16)
    for c in range(n_cores):
        o = 0
        kk = 0
        for hidx in range(len(halves)):
            for k in range(U[hidx] // 1024):
                vals = upad[c, o + k * 1024:o + (k + 1) * 1024] - BASES[kk]
                kk += 1
                pos = np.arange(1024)
                b16[c, pos % 16, (o + k * 1024 + pos) // 16] = \
                    vals.astype(np.int16)
            o += U[hidx]
    bfull = np.tile(b16, (1, 8, 1))
    # per-edge idx into its half's compact table; the build stores the
    # gathered block [p, j] at table row 1024k + 8p + j (p = pos%128,
    # j = pos//128 within the 1024-block)
    e_row = np.empty(E, np.int64)
    for c in range(n_cores):
        for hidx in range(len(halves)):
            m = (ec == c) & (e_half == hidx)
            pos = np.searchsorted(uniq[c][hidx], psrc[m])
            e_row[m] = (1024 * (pos // 1024) + 8 * (pos % 128)
                        + (pos % 1024) // 128)
    g16 = np.zeros((n_cores, 16, W * 8), np.int16)
    g16[ec, qrow, offs[et] * 8 + qcol] = e_row.astype(np.int16)
    deg_slot_v = np.zeros((n_cores, NT, 128), np.int64)
    deg_slot_v[core, ltile, slot] = deg
    gfull = np.tile(g16, (1, 8, 1))

    # per-slot degree [cores, 128, NT] (den base = deg*etc)
    degc = np.swapaxes(deg_slot_v.astype(np.float32), 1, 2)

    h_pad = np.zeros((NP, H), np.float32)
    h_pad[p_glob] = h

    apply_b1 = not np.allclose(np.asarray(b1), 0.0)
    apply_b2 = not np.allclose(np.asarray(b2), 0.0)
    apply_ln = not (np.allclose(np.asarray(ln_weight), 1.0)
                    and np.allclose(np.asarray(ln_bias), 0.0))

    cfg = Cfg(n_cores=n_cores, H=H, NT=NT, R=tuple(int(x) for x in R),
              SPLIT=SPLIT, apply_b1=apply_b1, apply_b2=apply_b2,
              apply_ln_affine=apply_ln, halves=halves,
              U=tuple(U), BASES=tuple(BASES))

    com = dict(
        bnw=np.asarray(bn_weight, np.float32).reshape(1, H),
        bnb=np.asarray(bn_bias, np.float32).reshape(1, H),
        bnm=np.asarray(bn_mean, np.float32).reshape(1, H),
        bnv=np.asarray(bn_var, np.float32).reshape(1, H),
        t_sc=np.asarray(t, np.float32).reshape(1, 1),
        W1=np.asarray(W1, np.float32),
        W2=np.asarray(W2, np.float32),
    )
    if apply_b1:
        com["b1"] = np.asarray(b1, np.float32).reshape(1, 2 * H)
    if apply_b2:
        com["b2"] = np.asarray(b2, np.float32).reshape(1, H)
    if apply_ln:
        com["lnw"] = np.asarray(ln_weight, np.float32).reshape(1, 2 * H)
        com["lnb"] = np.asarray(ln_bias, np.float32).reshape(1, 2 * H)

    in_maps = []
    for c in range(n_cores):
        m = dict(com)
        m["h_rows"] = np.ascontiguousarray(h_pad[c * NPC:(c + 1) * NPC])
        m["eattr"] = np.ascontiguousarray(EA[c])
        m["gidx"] = np.ascontiguousarray(gfull[c])
        m["bidx"] = np.ascontiguousarray(bfull[c])
        m["degc"] = np.ascontiguousarray(degc[c])
        in_maps.append(m)

    meta = dict(N=N, NPC=NPC, perm_pos=p_glob)
    return cfg, in_maps, meta


def assemble_output(results, meta):
    full = np.concatenate([r["out"] for r in results], axis=0)
    return full[meta["perm_pos"]].astype(np.float32)


_cache = {}


def kernel(**inputs):
    cfg, in_maps, meta = host_prep(**{k: np.asarray(v)
                                      for k, v in inputs.items()}, n_cores=8)
    key = (cfg.NT, cfg.R, cfg.U, cfg.BASES, cfg.apply_b1, cfg.apply_b2,
           cfg.apply_ln_affine)
    nc = _cache.get(key)
    if nc is None:
        nc = build_gcn(cfg)
        _cache[key] = nc

    from concourse.bass_utils import run_bass_kernel_spmd
    res = run_bass_kernel_spmd(nc, in_maps, core_ids=list(range(cfg.n_cores)))
    return assemble_output(res.results, meta)
